# revision 35
# baseline (speedup 1.0000x reference)
"""Trainium2 Bass kernel for nn_MixtureOfExperts (B=524288, IN=59, E=4, H=64).

Pure data parallel over 8 cores, 65536 rows each; per core 8 rounds of 8192
rows (16 tiles of 512).  Cost-model-driven design:

 - Host folds BN into weights, collapses the expert head w3@wp -> wep, folds
   |wep| into w2's columns (stage-3 reduction weights become exact +-1
   signs), folds layer-1/gate biases into the weights' ones-row, and ships x
   feature-major [64, BC] bf16 with a ones row at feature 59.
 - Per 512-row tile, all bf16 matmuls:
     stage1: 2 weights-stationary matmuls into one 2-bank psum [128,1024]
     gate hidden: 1 matmul into a 32-partition strip of a shared psum
     stage2: 2 block-diagonal K=128 matmuls (|wep|-scaled)
     stage3 preds / gate logits / bias: DATA-stationary matmuls (the relu'd
       activations are the stationary operand, tiny reduction weights
       stream) costing only N=2..8 moving columns each.  They accumulate
       batch-major into one psum "tail" bank per round: chunk cc of 128
       rows -> cols [8cc:8cc+8] = [p0 p1 p2 p3 l0 l1 l2 l3].
 - Relu passes (psum -> SBUF bf16): the merged h1 pass [128,1024] runs on
   Act (pure relu, biases pre-folded); the 32 h2 passes (per-partition bias
   APs) and 4 gate passes split Act/DVE by a 6/30 quota; the last round
   places its Act quota late so both engines finish together.  GPSIMD
   cannot touch PSUM (BIR verifier), so only SBUF-side tail math goes
   there.
 - Gate blocks lead their expert tiles by ~2 tiles so the tail bank's
   bias+logits land early; preds/bias/logits writers are capped at 256
   (walrus writer limit) by keeping one 8-wide bias matmul per chunk.
 - Round tail in two phases: phase 1 at round end exits psum (Act exp of
   logits + Act copy of preds) and sums p*exp / exp via pairwise adds on
   the otherwise-idle GPSIMD; phase 2 (DVE reciprocal, GPSIMD multiply,
   DMA out) is deferred two tiles into the NEXT round so the DVE's
   in-order queue never head-of-line blocks on the gpsimd chain.  The
   last round instead drains through four 16-chunk pieces issued as their
   tiles complete.
 - Cross-round software pipelining: each round issues the next round's
   first two stage-1s (and its x DMAs mid-round), so Act's h1 chain never
   breaks at round boundaries.  A gpsimd-memset-fed warmup matmul chain
   keeps PE's p-state ramp warm through the initial weight-DMA wait.

A round-0 prologue issues stage-1 of tiles 0/1 before the first gate
block, and the startup is latency-optimal: the per-core wb tensor embeds
x[:, 0:512] (cols X00:X00+512) so a single critical DMA delivers both the
stage-1 weights and the first x tile.

Engine busy (CoreSim): Act ~170us (bound, 95%), DVE ~160us, PE ~140us;
sim 178.5us (vs 181.3us prior, 294.5us naive).  Floor analysis: every
h1/h2/gate element must exit PSUM through Act or DVE at 1 col/cycle
(GPSIMD is verifier-blocked from PSUM, DMA cannot touch PSUM, matmul
PSUM stays f32 on TRN2), giving a ~129us processing floor + ~35us
bank-limited per-pass init overhead; the kernel sits within ~4% of that
structural bound.
"""

import numpy as np
import ml_dtypes

import concourse.bass as bass
import concourse.mybir as mybir
import concourse.tile as tile
from concourse import bacc
from concourse.bass_utils import run_bass_kernel_spmd

F32 = mybir.dt.float32
BF16 = mybir.dt.bfloat16
AF = mybir.ActivationFunctionType
ALU = mybir.AluOpType
AX = mybir.AxisListType

B, IN, E, H, EMB, GH = 524288, 59, 4, 64, 32, 32
EPS = 1e-5
NCORES = 8
BC = B // NCORES            # 65536 rows per core
NR = 8                      # rounds per core
RS = BC // NR               # 8192 rows per round
NT = RS // 512              # 16 tiles of 512 per round

# wb (bf16) column layout; cols X00:X00+512 hold the core's x[:, 0:512]
# so one DMA delivers both the stage-1 weights and the first x tile
W1A0, W1B0, X00, GW10, W2A0, W2B0 = 0, 128, 256, 768, 800, 928
SGA0, SGB0, GW2R0, BEP0 = 1056, 1058, 1060, 1064
WB_W = 1072
# wf (f32) column layout: c1a c1b c2a c2b gb1t
WF_W = 8

_CACHE = {}

# relu engine assignment: per 16-tile round there are 68 psum->SBUF relu
# passes (64 tile + 4 gate).  GPSIMD cannot touch PSUM (BIR verifier), so
# they split across Act/DVE; Act is slightly faster per pass but also runs
# the exp, DVE runs the reductions/reciprocal.
def _relu_engines(total=36, quota=None):
    quota = quota or {"act": 5, "dve": 30}
    order = []
    frac = {k: 0.0 for k in quota}
    for _ in range(total):
        for k in frac:
            frac[k] += quota[k] / total
        pick = max(frac, key=lambda k: frac[k])
        frac[pick] -= 1.0
        order.append(pick)
    return order

RELU_ENG = _relu_engines(36, {"act": 6, "dve": 30})
# last round: act quota placed late (tiles ~9-15) so Act and DVE finish
# together and the drain isn't DVE-bound
RELU_LAST = ["dve"] * 20 + _relu_engines(16, {"act": 7, "dve": 9})
ROUND_ENG = None            # optional per-round list of 36-slot patterns
# tail mode per round: "pcopy_act" / "pcopy_dve" (copy + gpsimd mul) or
# "dve_tt" (DVE multiplies psum*exp directly, no copy)
TAIL_MODES = ["pcopy_act"] * 8
# last round tail pieces: (issue_after_tile, flush_after_tile, lo, hi, mode)
LAST_PIECES = [(3, 5, 0, 16, None), (7, 9, 16, 32, None),
               (11, 13, 32, 48, None), (15, None, 48, 64, None)]
H1_PATTERN = ["act"] * 16   # per-tile engine for the h1 relu pass


def _build():
    nc = bacc.Bacc(trn_type="TRN2")
    x_d = nc.dram_tensor("x", (64, BC), BF16, kind="ExternalInput")
    wb_d = nc.dram_tensor("wb", (128, WB_W), BF16, kind="ExternalInput")
    wf_d = nc.dram_tensor("wf", (128, WF_W), F32, kind="ExternalInput")
    out_d = nc.dram_tensor("out", (NR, 128, 64), F32, kind="ExternalOutput")

    with tile.TileContext(nc) as tc:
        with (
            tc.tile_pool(name="consts", bufs=1) as consts,
            tc.tile_pool(name="xp", bufs=2) as xp,
            tc.tile_pool(name="h1p", bufs=4) as h1p,
            tc.tile_pool(name="h2p", bufs=6) as h2p,
            tc.tile_pool(name="g1p", bufs=3) as g1p,
            tc.tile_pool(name="tp", bufs=2) as tp,
            tc.tile_pool(name="p1", bufs=2, space="PSUM") as p1p,
            tc.tile_pool(name="p2a", bufs=1, space="PSUM") as p2ap,
            tc.tile_pool(name="p2b", bufs=1, space="PSUM") as p2bp,
            tc.tile_pool(name="pga", bufs=1, space="PSUM") as pgap,
            tc.tile_pool(name="ptl", bufs=1, space="PSUM") as ptlp,
        ):
            # startup criticals first: x cols 0:512 and the stage-1
            # weights, so the round-0 prologue's first matmul fires ASAP
            x0_sb = xp.tile([64, RS], BF16, tag="x")
            wb = consts.tile([128, WB_W], BF16)
            nc.sync.dma_start(out=wb[:, 0:768], in_=wb_d[:, 0:768])
            # PE p-state warmup: keep PE continuously busy through the
            # initial DMA wait so the first real matmuls run at mid/full
            # clock instead of cold (1.54ns/col).
            warm = consts.tile([64, 512], BF16)
            nc.gpsimd.memset(warm, 0.0)
            wps = pgap.tile([128, 512], F32, tag="ga")
            for _ in range(4):
                nc.tensor.matmul(
                    out=wps[0:64, :], lhsT=warm[:, 0:64], rhs=warm,
                    start=True, stop=True, skip_group_check=True)
            cw0 = RS // 4
            # startup order tuned for the dependency chain: tile-1 x first
            # (stage1(1)), then the stage-2/gate weights, then tiles 2-3
            nc.sync.dma_start(out=x0_sb[:, 512:1024], in_=x_d[:, 512:1024])
            nc.sync.dma_start(out=wb[:, 768:WB_W], in_=wb_d[:, 768:WB_W])
            wf = consts.tile([128, WF_W], F32)
            nc.sync.dma_start(out=wf, in_=wf_d[:, :])
            nc.sync.dma_start(out=x0_sb[:, 1024:cw0], in_=x_d[:, 1024:cw0])

            w1a = wb[0:64, W1A0:W1A0 + 128]
            w1b = wb[0:64, W1B0:W1B0 + 128]
            gw1 = wb[0:64, GW10:GW10 + 32]
            w2a = wb[:, W2A0:W2A0 + 128]
            w2b = wb[:, W2B0:W2B0 + 128]
            sga = wb[:, SGA0:SGA0 + 2]
            sgb = wb[:, SGB0:SGB0 + 2]
            gw2r = wb[:, GW2R0:GW2R0 + 4]
            bep8 = wb[0:64, BEP0:BEP0 + 8]
            c2a = wf[:, 2:3]
            c2b = wf[:, 3:4]

            pending = []

            def relu(eng, out_sb, in_ps, bias_ap):
                if eng == "act":
                    nc.scalar.activation(
                        out_sb, in_ps, AF.Relu,
                        bias=bias_ap if bias_ap is not None else 0.0)
                elif bias_ap is None:
                    nc.vector.tensor_scalar(
                        out_sb, in_ps, 0.0, None, ALU.max)
                else:
                    nc.vector.tensor_scalar(
                        out_sb, in_ps, bias_ap, 0.0, ALU.add, ALU.max)

            def issue_x_dmas(r, x_sb):
                cw = RS // 4
                for ch in range(4):
                    if r == 0 and ch == 0:
                        continue
                    nc.sync.dma_start(
                        out=x_sb[:, ch * cw:(ch + 1) * cw],
                        in_=x_d[:, r * RS + ch * cw: r * RS + (ch + 1) * cw])

            issue_x_dmas(0, x0_sb)
            x_cur = x0_sb
            carry = None  # (h1r_t0, h1r_t1) pipelined from the prev round

            for r in range(NR):
                x_sb = x_cur

                tail = ptlp.tile([128, 512], F32, tag="tail")
                ri = 0  # relu slot index within round
                reng = (ROUND_ENG[r] if ROUND_ENG is not None
                        else (RELU_LAST if r == NR - 1 else RELU_ENG))

                def xsrc(t, lo, hi, xs=None):
                    if xs is None and r == 0 and t == 0:
                        return wb[0:64, X00 + lo:X00 + hi]
                    return (xs if xs is not None
                            else x_sb)[:, 512 * t + lo:512 * t + hi]

                def gate_block(g, first):
                    ga = pgap.tile([128, 512], F32, tag="ga")
                    g1r = g1p.tile([128, 512], BF16, tag="g1r")
                    for i in range(4):
                        t = 4 * g + i
                        nc.tensor.matmul(
                            out=ga[32 * i:32 * i + 32, :], lhsT=gw1,
                            rhs=xsrc(t, 0, 512),
                            start=True, stop=True, skip_group_check=True,
                            tile_position=(0, 32 * i))
                    nonlocal ri
                    relu(reng[ri], g1r, ga, None); ri += 1
                    for i in range(4):
                        t = 4 * g + i
                        for q in range(4):
                            col = 8 * (4 * t + q)
                            xq = xsrc(t, 128 * q, 128 * q + 128)
                            # bias row (bep, gb2): fresh write of the chunk's
                            # 8 cols; logits/preds accumulate on top.
                            nc.tensor.matmul(
                                out=tail[:, col:col + 8],
                                lhsT=xq, rhs=bep8,
                                start=(first and i == 0 and q == 0),
                                stop=False,
                                skip_group_check=True)
                            nc.tensor.matmul(
                                out=tail[:, col + 4:col + 8],
                                lhsT=g1r[32 * i:32 * i + 32,
                                         128 * q:128 * q + 128],
                                rhs=gw2r[32 * i:32 * i + 32, :],
                                start=False, stop=False,
                                skip_group_check=True,
                                tile_position=(32 * i, 0))

                def stage1(t, eng="act", xs_buf=None):
                    xs = xsrc(t, 0, 512, xs_buf)
                    p1 = p1p.tile([128, 1024], F32, tag="p1")
                    nc.tensor.matmul(
                        out=p1[:, 0:512], lhsT=w1a, rhs=xs,
                        start=True, stop=True, skip_group_check=True)
                    nc.tensor.matmul(
                        out=p1[:, 512:1024], lhsT=w1b, rhs=xs,
                        start=True, stop=True, skip_group_check=True)
                    h1r = h1p.tile([128, 1024], BF16, tag="h1r")
                    relu(eng, h1r, p1, None)
                    return h1r

                def stage2relu(t, h1r):
                    nonlocal ri
                    p2a = p2ap.tile([128, 512], F32, tag="p2a")
                    nc.tensor.matmul(
                        out=p2a, lhsT=w2a, rhs=h1r[:, 0:512],
                        start=True, stop=True)
                    p2b = p2bp.tile([128, 512], F32, tag="p2b")
                    nc.tensor.matmul(
                        out=p2b, lhsT=w2b, rhs=h1r[:, 512:1024],
                        start=True, stop=True)
                    h2ra = h2p.tile([128, 512], BF16, tag="h2ra")
                    relu(reng[ri], h2ra, p2a, c2a); ri += 1
                    h2rb = h2p.tile([128, 512], BF16, tag="h2rb")
                    relu(reng[ri], h2rb, p2b, c2b); ri += 1
                    return h2ra, h2rb

                def preds_block(t, h2ra, h2rb):
                    for q in range(4):
                        col = 8 * (4 * t + q)
                        nc.tensor.matmul(
                            out=tail[:, col:col + 2],
                            lhsT=h2ra[:, 128 * q:128 * q + 128],
                            rhs=sga,
                            start=False, stop=False,
                            skip_group_check=True)
                        nc.tensor.matmul(
                            out=tail[:, col + 2:col + 4],
                            lhsT=h2rb[:, 128 * q:128 * q + 128],
                            rhs=sgb,
                            start=False,
                            stop=(t == NT - 1 and q == 3),
                            skip_group_check=True)

                # ---- round tail: combine (batch-major) over chunk range.
                # Act/DVE only exit PSUM (exp + preds copy); the arithmetic
                # (mul, pairwise-add reductions) runs on the otherwise-idle
                # GPSIMD.  phase2 (recip on DVE + final mul + DMA) is issued
                # a couple of tiles into the NEXT round so the DVE's in-order
                # queue never stalls waiting on the gpsimd chain.
                def round_tail(lo, hi, suf, rr, mode=None):
                    n = hi - lo
                    tv = tail.rearrange("p (c k) -> p c k", k=8)[:, lo:hi]
                    expl = tp.tile([128, n * 4], F32, tag="expl" + suf)
                    ev = expl.rearrange("p (c k) -> p c k", k=4)
                    nc.scalar.activation(ev, tv[:, :, 4:8], AF.Exp)
                    mode = mode or TAIL_MODES[rr]
                    w_sb = tp.tile([128, n * 4], F32, tag="w" + suf)
                    wv = w_sb.rearrange("p (c k) -> p c k", k=4)
                    if mode == "dve_drain":
                        # final drain piece: everything on DVE serially --
                        # avoids six gpsimd Q7-launch overheads on the
                        # kernel's critical exit path
                        nc.vector.tensor_mul(wv, tv[:, :, 0:4], ev)
                        num = tp.tile([128, n], F32, tag="num" + suf)
                        nc.vector.tensor_reduce(num, wv, AX.X, ALU.add)
                        den = tp.tile([128, n], F32, tag="den" + suf)
                        nc.vector.tensor_reduce(den, ev, AX.X, ALU.add)
                        rec = tp.tile([128, n], F32, tag="rec" + suf)
                        nc.vector.reciprocal(rec, den)
                        o_sb = tp.tile([128, n], F32, tag="o" + suf)
                        nc.vector.tensor_mul(o_sb, num, rec)
                        nc.sync.dma_start(out=out_d[rr][:, lo:hi], in_=o_sb)
                        return lambda: None
                    if mode == "dve_tt":
                        nc.vector.tensor_mul(wv, tv[:, :, 0:4], ev)
                    else:
                        p_sb = tp.tile([128, n * 4], F32, tag="p" + suf)
                        pv = p_sb.rearrange("p (c k) -> p c k", k=4)
                        if mode == "pcopy_dve":
                            nc.vector.tensor_copy(pv, tv[:, :, 0:4])
                        else:
                            nc.scalar.copy(pv, tv[:, :, 0:4])
                        nc.gpsimd.tensor_mul(w_sb, p_sb, expl)
                    # 4-wide sums as pairwise adds (gpsimd reduce lacks X axis)
                    ws = tp.tile([128, n * 2], F32, tag="ws" + suf)
                    wsv = ws.rearrange("p (c k) -> p c k", k=2)
                    nc.gpsimd.tensor_add(wsv, wv[:, :, 0:2], wv[:, :, 2:4])
                    num = tp.tile([128, n], F32, tag="num" + suf)
                    nc.gpsimd.tensor_add(num, wsv[:, :, 0], wsv[:, :, 1])
                    es = tp.tile([128, n * 2], F32, tag="es" + suf)
                    esv = es.rearrange("p (c k) -> p c k", k=2)
                    nc.gpsimd.tensor_add(esv, ev[:, :, 0:2], ev[:, :, 2:4])
                    den = tp.tile([128, n], F32, tag="den" + suf)
                    nc.gpsimd.tensor_add(den, esv[:, :, 0], esv[:, :, 1])

                    def phase2():
                        rec = tp.tile([128, n], F32, tag="rec" + suf)
                        nc.vector.reciprocal(rec, den)
                        o_sb = tp.tile([128, n], F32, tag="o" + suf)
                        nc.gpsimd.tensor_mul(o_sb, num, rec)
                        nc.sync.dma_start(out=out_d[rr][:, lo:hi], in_=o_sb)
                    return phase2

                # staggered interleave: gate(g) leads its expert tiles by
                # ~2 tiles so logits/bias are ready early without starving
                # Act/DVE at round starts.
                def expert_tile(t):
                    preds_block(t, *stage2relu(t, stage1(t, H1_PATTERN[t])))

                def flush_pending():
                    for p2 in pending:
                        p2()
                    pending.clear()

                last = r == NR - 1
                if carry is None:
                    h1r_t0 = stage1(0, H1_PATTERN[0])
                    h1r_t1 = stage1(1, "dve")
                else:
                    h1r_t0, h1r_t1 = carry
                gate_block(0, True)
                preds_block(0, *stage2relu(0, h1r_t0))
                preds_block(1, *stage2relu(1, h1r_t1))
                gate_block(1, False)
                expert_tile(2)
                flush_pending()  # prev round's tail phase2 (data long ready)
                # LAST_PIECES: (issue_after_tile, flush_after_tile, lo, hi,
                # mode) for the final round's tail pieces
                lp = {}
                if last:
                    for i, (it, ft, lo, hi, md) in enumerate(LAST_PIECES):
                        lp.setdefault(it, []).append(("issue", lo, hi, i, md))
                        if ft is not None:
                            lp.setdefault(ft, []).append(("flush",))

                def tile_hooks(t):
                    for h in lp.get(t, ()):
                        if h[0] == "issue":
                            _, lo, hi, i, md = h
                            p2 = round_tail(lo, hi, "p%d" % i, r, mode=md)
                            if md == "dve_drain":
                                pass
                            else:
                                pending.append(p2)
                        else:
                            flush_pending()

                expert_tile(3); tile_hooks(3)
                expert_tile(4); tile_hooks(4)
                expert_tile(5); tile_hooks(5)
                gate_block(2, False)
                expert_tile(6); tile_hooks(6)
                # prefetch next round's x while this round is mid-flight
                if r + 1 < NR:
                    x_next = xp.tile([64, RS], BF16, tag="x")
                    issue_x_dmas(r + 1, x_next)
                expert_tile(7); tile_hooks(7)
                expert_tile(8); tile_hooks(8)
                expert_tile(9); tile_hooks(9)
                gate_block(3, False)
                expert_tile(10); tile_hooks(10)
                expert_tile(11); tile_hooks(11)
                expert_tile(12); tile_hooks(12)
                expert_tile(13); tile_hooks(13)
                expert_tile(14); tile_hooks(14)
                expert_tile(15); tile_hooks(15)
                if last:
                    flush_pending()
                else:
                    pending.append(round_tail(0, 64, "", r))
                    # software-pipeline the next round's first two stage-1s
                    # so Act never idles across the round boundary
                    nxt0 = stage1(0, H1_PATTERN[0], xs_buf=x_next)
                    nxt1 = stage1(1, H1_PATTERN[1], xs_buf=x_next)
                    carry = (nxt0, nxt1)
                    x_cur = x_next



    if not nc.is_finalized():
        nc.finalize()
    return nc


def _pack_host(w1, b1, bn1_g, bn1_b, bn1_m, bn1_v, w2, b2, bn2_g, bn2_b,
               bn2_m, bn2_v, w3, b3, wp, bp, gw1, gb1, gw2, gb2):
    f = np.float32
    s1 = (bn1_g / np.sqrt(bn1_v + EPS)).astype(f)
    w1e = (w1 * s1[:, None, :]).astype(f)                       # (E,IN,H)
    c1 = ((b1 - bn1_m) * s1 + bn1_b).astype(f)                  # (E,H)
    s2 = (bn2_g / np.sqrt(bn2_v + EPS)).astype(f)
    w2e = (w2 * s2[:, None, :]).astype(f)                       # (E,H,H)
    c2 = ((b2 - bn2_m) * s2 + bn2_b).astype(f)                  # (E,H)
    wep = np.einsum("ehm,em->eh", w3, wp).astype(f)             # (E,H)
    bep = (np.einsum("em,em->e", b3, wp) + bp).astype(f)        # (E,)

    aw = np.abs(wep)                                            # (E,H)
    sg = np.sign(wep).astype(f)
    w2f = w2e * aw[:, None, :]                                  # cols scaled
    c2f = c2 * aw

    wb = np.zeros((128, WB_W), f)  # cols X00:X00+512 filled per core
    wb[0:IN, W1A0:W1A0 + 64] = w1e[0]
    wb[0:IN, W1A0 + 64:W1A0 + 128] = w1e[1]
    wb[0:IN, W1B0:W1B0 + 64] = w1e[2]
    wb[0:IN, W1B0 + 64:W1B0 + 128] = w1e[3]
    wb[IN, W1A0:W1A0 + 128] = np.concatenate([c1[0], c1[1]])
    wb[IN, W1B0:W1B0 + 128] = np.concatenate([c1[2], c1[3]])
    wb[0:IN, GW10:GW10 + 32] = gw1
    wb[IN, GW10:GW10 + 32] = gb1
    wb[0:64, W2A0:W2A0 + 64] = w2f[0]
    wb[64:128, W2A0 + 64:W2A0 + 128] = w2f[1]
    wb[0:64, W2B0:W2B0 + 64] = w2f[2]
    wb[64:128, W2B0 + 64:W2B0 + 128] = w2f[3]
    wb[0:64, SGA0] = sg[0]
    wb[64:128, SGA0 + 1] = sg[1]
    wb[0:64, SGB0] = sg[2]
    wb[64:128, SGB0 + 1] = sg[3]
    for gi in range(4):
        wb[32 * gi:32 * gi + 32, GW2R0:GW2R0 + 4] = gw2
    wb[IN, BEP0:BEP0 + 4] = bep
    wb[IN, BEP0 + 4:BEP0 + 8] = gb2

    wf = np.zeros((128, WF_W), f)
    wf[:, 2] = np.concatenate([c2f[0], c2f[1]])
    wf[:, 3] = np.concatenate([c2f[2], c2f[3]])
    return dict(wb=wb.astype(ml_dtypes.bfloat16), wf=wf)


def _x_core(xc):
    """(BC, 59) f32 -> (64, BC) bf16 feature-major with ones row at 59."""
    xt = np.zeros((64, BC), np.float32)
    xt[:IN] = xc.T
    xt[IN] = 1.0
    return np.ascontiguousarray(xt).astype(ml_dtypes.bfloat16)


def _unpack_out(o):
    """(NR, 128, 64) -> (BC,): row = 8192 r + 512 t + 128 q + b, col=4t+q."""
    o = np.asarray(o, np.float32).reshape(NR, 128, NT, 4)
    return np.ascontiguousarray(o.transpose(0, 2, 3, 1)).reshape(BC)


def _core_maps(xt, packed):
    wbc = packed["wb"].copy()
    wbc[0:64, X00:X00 + 512] = xt[:, 0:512]
    return {"x": xt, "wb": wbc, "wf": packed["wf"]}


def _sim_inputs(x_full, packed):
    return _core_maps(_x_core(np.asarray(x_full, np.float32)[:BC]), packed)


def kernel(**inputs):
    x = np.asarray(inputs["x"], dtype=np.float32)
    wk = {k: np.asarray(v, dtype=np.float32) for k, v in inputs.items()
          if k != "x"}
    packed = _pack_host(**wk)

    if "nc" not in _CACHE:
        _CACHE["nc"] = _build()
    nc = _CACHE["nc"]

    in_maps = [_core_maps(_x_core(x[c * BC:(c + 1) * BC]), packed)
               for c in range(NCORES)]

    res = run_bass_kernel_spmd(nc, in_maps, core_ids=list(range(NCORES)))
    _CACHE["last"] = res
    outs = [_unpack_out(r["out"]) for r in res.results]
    return np.concatenate(outs).reshape(B, 1).astype(np.float32)



# revision 39
# speedup vs baseline: 1.0000x; 1.0000x over previous
"""Trainium2 Bass kernel for nn_MixtureOfExperts (B=524288, IN=59, E=4, H=64).

Pure data parallel over 8 cores, 65536 rows each; per core 8 rounds of 8192
rows (16 tiles of 512).  Cost-model-driven design:

 - Host folds BN into weights, collapses the expert head w3@wp -> wep, folds
   |wep| into w2's columns (stage-3 reduction weights become exact +-1
   signs), folds layer-1/gate biases into the weights' ones-row, and ships x
   feature-major [64, BC] bf16 with a ones row at feature 59.
 - Per 512-row tile, all bf16 matmuls:
     stage1: 2 weights-stationary matmuls into one 2-bank psum [128,1024]
     gate hidden: 1 matmul into a 32-partition strip of a shared psum
     stage2: 2 block-diagonal K=128 matmuls (|wep|-scaled)
     stage3 preds / gate logits / bias: DATA-stationary matmuls (the relu'd
       activations are the stationary operand, tiny reduction weights
       stream) costing only N=2..8 moving columns each.  They accumulate
       batch-major into one psum "tail" bank per round: chunk cc of 128
       rows -> cols [8cc:8cc+8] = [p0 p1 p2 p3 l0 l1 l2 l3].
 - Relu passes (psum -> SBUF bf16): the merged h1 pass [128,1024] runs on
   Act (pure relu, biases pre-folded); the 32 h2 passes (per-partition bias
   APs) and 4 gate passes split Act/DVE by a 6/30 quota; the last round
   places its Act quota late so both engines finish together.  GPSIMD
   cannot touch PSUM (BIR verifier), so only SBUF-side tail math goes
   there.
 - Gate blocks lead their expert tiles by ~2 tiles so the tail bank's
   bias+logits land early; preds/bias/logits writers are capped at 256
   (walrus writer limit) by keeping one 8-wide bias matmul per chunk.
 - Round tail in two phases: phase 1 at round end exits psum (Act exp of
   logits + Act copy of preds) and sums p*exp / exp via pairwise adds on
   the otherwise-idle GPSIMD; phase 2 (DVE reciprocal, GPSIMD multiply,
   DMA out) is deferred two tiles into the NEXT round so the DVE's
   in-order queue never head-of-line blocks on the gpsimd chain.  The
   last round instead drains through four 16-chunk pieces issued as their
   tiles complete.
 - Cross-round software pipelining: each round issues the next round's
   first two stage-1s (and its x DMAs mid-round), so Act's h1 chain never
   breaks at round boundaries.  A gpsimd-memset-fed warmup matmul chain
   keeps PE's p-state ramp warm through the initial weight-DMA wait.

A round-0 prologue issues stage-1 of tiles 0/1 before the first gate
block, and the startup is latency-optimal: the per-core wb tensor embeds
x[:, 0:512] (cols X00:X00+512) so a single critical DMA delivers both the
stage-1 weights and the first x tile.

Engine busy (CoreSim): Act ~170us (bound, 95%), DVE ~160us, PE ~140us;
sim 178.5us (vs 181.3us prior, 294.5us naive).  Floor analysis: every
h1/h2/gate element must exit PSUM through Act or DVE at 1 col/cycle
(GPSIMD is verifier-blocked from PSUM, DMA cannot touch PSUM, matmul
PSUM stays f32 on TRN2), giving a ~129us processing floor + ~35us
bank-limited per-pass init overhead; the kernel sits within ~4% of that
structural bound.
"""

import numpy as np
import ml_dtypes

import concourse.bass as bass
import concourse.mybir as mybir
import concourse.tile as tile
from concourse import bacc
from concourse.bass_utils import run_bass_kernel_spmd

F32 = mybir.dt.float32
BF16 = mybir.dt.bfloat16
AF = mybir.ActivationFunctionType
ALU = mybir.AluOpType
AX = mybir.AxisListType

B, IN, E, H, EMB, GH = 524288, 59, 4, 64, 32, 32
EPS = 1e-5
NCORES = 8
BC = B // NCORES            # 65536 rows per core
NR = 8                      # rounds per core
RS = BC // NR               # 8192 rows per round
NT = RS // 512              # 16 tiles of 512 per round

# wb (bf16) column layout; cols X00:X00+512 hold the core's x[:, 0:512]
# so one DMA delivers both the stage-1 weights and the first x tile
W1A0, W1B0, X00, GW10, W2A0, W2B0 = 0, 128, 256, 768, 800, 928
SGA0, SGB0, GW2R0, BEP0 = 1056, 1058, 1060, 1064
WB_W = 1072
# wf (f32) column layout: c1a c1b c2a c2b gb1t
WF_W = 8

_CACHE = {}

# relu engine assignment: per 16-tile round there are 68 psum->SBUF relu
# passes (64 tile + 4 gate).  GPSIMD cannot touch PSUM (BIR verifier), so
# they split across Act/DVE; Act is slightly faster per pass but also runs
# the exp, DVE runs the reductions/reciprocal.
def _relu_engines(total=36, quota=None):
    quota = quota or {"act": 5, "dve": 30}
    order = []
    frac = {k: 0.0 for k in quota}
    for _ in range(total):
        for k in frac:
            frac[k] += quota[k] / total
        pick = max(frac, key=lambda k: frac[k])
        frac[pick] -= 1.0
        order.append(pick)
    return order

RELU_ENG = _relu_engines(36, {"act": 6, "dve": 30})
# last round: act quota placed late (tiles ~9-15) so Act and DVE finish
# together and the drain isn't DVE-bound
RELU_LAST = ["dve"] * 20 + _relu_engines(16, {"act": 7, "dve": 9})
ROUND_ENG = None            # optional per-round list of 36-slot patterns
# tail mode per round: "pcopy_act" / "pcopy_dve" (copy + gpsimd mul) or
# "dve_tt" (DVE multiplies psum*exp directly, no copy)
TAIL_MODES = ["pcopy_act"] * 8
# last round tail pieces: (issue_after_tile, flush_after_tile, lo, hi, mode)
LAST_PIECES = [(3, 5, 0, 16, None), (7, 9, 16, 32, None),
               (11, 13, 32, 48, None), (14, 15, 48, 60, None),
               (15, None, 60, 64, None)]
H1_PATTERN = ["act"] * 16   # per-tile engine for the h1 relu pass
FLUSH_TILE = 2              # tile after which prev-round phase2 is flushed
XPF_TILE = 6                # tile after which next round's x DMAs are issued


def _build():
    nc = bacc.Bacc(trn_type="TRN2")
    x_d = nc.dram_tensor("x", (64, BC), BF16, kind="ExternalInput")
    wb_d = nc.dram_tensor("wb", (128, WB_W), BF16, kind="ExternalInput")
    wf_d = nc.dram_tensor("wf", (128, WF_W), F32, kind="ExternalInput")
    out_d = nc.dram_tensor("out", (NR, 128, 64), F32, kind="ExternalOutput")

    with tile.TileContext(nc) as tc:
        with (
            tc.tile_pool(name="consts", bufs=1) as consts,
            tc.tile_pool(name="xp", bufs=2) as xp,
            tc.tile_pool(name="h1p", bufs=4) as h1p,
            tc.tile_pool(name="h2p", bufs=6) as h2p,
            tc.tile_pool(name="g1p", bufs=3) as g1p,
            tc.tile_pool(name="tp", bufs=2) as tp,
            tc.tile_pool(name="p1", bufs=2, space="PSUM") as p1p,
            tc.tile_pool(name="p2a", bufs=1, space="PSUM") as p2ap,
            tc.tile_pool(name="p2b", bufs=1, space="PSUM") as p2bp,
            tc.tile_pool(name="pga", bufs=1, space="PSUM") as pgap,
            tc.tile_pool(name="ptl", bufs=1, space="PSUM") as ptlp,
        ):
            # startup criticals first: x cols 0:512 and the stage-1
            # weights, so the round-0 prologue's first matmul fires ASAP
            x0_sb = xp.tile([64, RS], BF16, tag="x")
            wb = consts.tile([128, WB_W], BF16)
            nc.sync.dma_start(out=wb[:, 0:768], in_=wb_d[:, 0:768])
            # PE p-state warmup: keep PE continuously busy through the
            # initial DMA wait so the first real matmuls run at mid/full
            # clock instead of cold (1.54ns/col).
            warm = consts.tile([64, 512], BF16)
            nc.gpsimd.memset(warm, 0.0)
            wps = pgap.tile([128, 512], F32, tag="ga")
            for _ in range(4):
                nc.tensor.matmul(
                    out=wps[0:64, :], lhsT=warm[:, 0:64], rhs=warm,
                    start=True, stop=True, skip_group_check=True)
            cw0 = RS // 4
            # startup order tuned for the dependency chain: tile-1 x first
            # (stage1(1)), then the stage-2/gate weights, then tiles 2-3
            nc.sync.dma_start(out=x0_sb[:, 512:1024], in_=x_d[:, 512:1024])
            nc.sync.dma_start(out=wb[:, 768:WB_W], in_=wb_d[:, 768:WB_W])
            wf = consts.tile([128, WF_W], F32)
            nc.sync.dma_start(out=wf, in_=wf_d[:, :])
            nc.sync.dma_start(out=x0_sb[:, 1024:cw0], in_=x_d[:, 1024:cw0])

            w1a = wb[0:64, W1A0:W1A0 + 128]
            w1b = wb[0:64, W1B0:W1B0 + 128]
            gw1 = wb[0:64, GW10:GW10 + 32]
            w2a = wb[:, W2A0:W2A0 + 128]
            w2b = wb[:, W2B0:W2B0 + 128]
            sga = wb[:, SGA0:SGA0 + 2]
            sgb = wb[:, SGB0:SGB0 + 2]
            gw2r = wb[:, GW2R0:GW2R0 + 4]
            bep8 = wb[0:64, BEP0:BEP0 + 8]
            c2a = wf[:, 2:3]
            c2b = wf[:, 3:4]

            pending = []

            def relu(eng, out_sb, in_ps, bias_ap):
                if eng == "act":
                    nc.scalar.activation(
                        out_sb, in_ps, AF.Relu,
                        bias=bias_ap if bias_ap is not None else 0.0)
                elif bias_ap is None:
                    nc.vector.tensor_scalar(
                        out_sb, in_ps, 0.0, None, ALU.max)
                else:
                    nc.vector.tensor_scalar(
                        out_sb, in_ps, bias_ap, 0.0, ALU.add, ALU.max)

            def issue_x_dmas(r, x_sb):
                cw = RS // 4
                for ch in range(4):
                    if r == 0 and ch == 0:
                        continue
                    nc.sync.dma_start(
                        out=x_sb[:, ch * cw:(ch + 1) * cw],
                        in_=x_d[:, r * RS + ch * cw: r * RS + (ch + 1) * cw])

            issue_x_dmas(0, x0_sb)
            x_cur = x0_sb
            carry = None  # (h1r_t0, h1r_t1) pipelined from the prev round

            for r in range(NR):
                x_sb = x_cur

                tail = ptlp.tile([128, 512], F32, tag="tail")
                ri = 0  # relu slot index within round
                reng = (ROUND_ENG[r] if ROUND_ENG is not None
                        else (RELU_LAST if r == NR - 1 else RELU_ENG))

                def xsrc(t, lo, hi, xs=None):
                    if xs is None and r == 0 and t == 0:
                        return wb[0:64, X00 + lo:X00 + hi]
                    return (xs if xs is not None
                            else x_sb)[:, 512 * t + lo:512 * t + hi]

                def gate_block(g, first):
                    ga = pgap.tile([128, 512], F32, tag="ga")
                    g1r = g1p.tile([128, 512], BF16, tag="g1r")
                    for i in range(4):
                        t = 4 * g + i
                        nc.tensor.matmul(
                            out=ga[32 * i:32 * i + 32, :], lhsT=gw1,
                            rhs=xsrc(t, 0, 512),
                            start=True, stop=True, skip_group_check=True,
                            tile_position=(0, 32 * i))
                    nonlocal ri
                    relu(reng[ri], g1r, ga, None); ri += 1
                    for i in range(4):
                        t = 4 * g + i
                        for q in range(4):
                            col = 8 * (4 * t + q)
                            xq = xsrc(t, 128 * q, 128 * q + 128)
                            # bias row (bep, gb2): fresh write of the chunk's
                            # 8 cols; logits/preds accumulate on top.
                            nc.tensor.matmul(
                                out=tail[:, col:col + 8],
                                lhsT=xq, rhs=bep8,
                                start=(first and i == 0 and q == 0),
                                stop=False,
                                skip_group_check=True)
                            nc.tensor.matmul(
                                out=tail[:, col + 4:col + 8],
                                lhsT=g1r[32 * i:32 * i + 32,
                                         128 * q:128 * q + 128],
                                rhs=gw2r[32 * i:32 * i + 32, :],
                                start=False, stop=False,
                                skip_group_check=True,
                                tile_position=(32 * i, 0))

                def stage1(t, eng="act", xs_buf=None):
                    xs = xsrc(t, 0, 512, xs_buf)
                    p1 = p1p.tile([128, 1024], F32, tag="p1")
                    nc.tensor.matmul(
                        out=p1[:, 0:512], lhsT=w1a, rhs=xs,
                        start=True, stop=True, skip_group_check=True)
                    nc.tensor.matmul(
                        out=p1[:, 512:1024], lhsT=w1b, rhs=xs,
                        start=True, stop=True, skip_group_check=True)
                    h1r = h1p.tile([128, 1024], BF16, tag="h1r")
                    relu(eng, h1r, p1, None)
                    return h1r

                def stage2relu(t, h1r):
                    nonlocal ri
                    p2a = p2ap.tile([128, 512], F32, tag="p2a")
                    nc.tensor.matmul(
                        out=p2a, lhsT=w2a, rhs=h1r[:, 0:512],
                        start=True, stop=True)
                    p2b = p2bp.tile([128, 512], F32, tag="p2b")
                    nc.tensor.matmul(
                        out=p2b, lhsT=w2b, rhs=h1r[:, 512:1024],
                        start=True, stop=True)
                    h2ra = h2p.tile([128, 512], BF16, tag="h2ra")
                    relu(reng[ri], h2ra, p2a, c2a); ri += 1
                    h2rb = h2p.tile([128, 512], BF16, tag="h2rb")
                    relu(reng[ri], h2rb, p2b, c2b); ri += 1
                    return h2ra, h2rb

                def preds_block(t, h2ra, h2rb):
                    for q in range(4):
                        col = 8 * (4 * t + q)
                        nc.tensor.matmul(
                            out=tail[:, col:col + 2],
                            lhsT=h2ra[:, 128 * q:128 * q + 128],
                            rhs=sga,
                            start=False, stop=False,
                            skip_group_check=True)
                        nc.tensor.matmul(
                            out=tail[:, col + 2:col + 4],
                            lhsT=h2rb[:, 128 * q:128 * q + 128],
                            rhs=sgb,
                            start=False,
                            stop=(t == NT - 1 and q == 3),
                            skip_group_check=True)

                # ---- round tail: combine (batch-major) over chunk range.
                # Act/DVE only exit PSUM (exp + preds copy); the arithmetic
                # (mul, pairwise-add reductions) runs on the otherwise-idle
                # GPSIMD.  phase2 (recip on DVE + final mul + DMA) is issued
                # a couple of tiles into the NEXT round so the DVE's in-order
                # queue never stalls waiting on the gpsimd chain.
                def round_tail(lo, hi, suf, rr, mode=None):
                    n = hi - lo
                    tv = tail.rearrange("p (c k) -> p c k", k=8)[:, lo:hi]
                    mode = mode or TAIL_MODES[rr]
                    if mode == "poly":
                        # ONE contiguous Act copy exits the whole tail slice;
                        # exp(l) is a 3rd-order Horner polynomial on the idle
                        # GPSIMD (|l| <~ 0.4, err < 1e-3 abs), saving the
                        # separate strided Act exp + pcopy pair.
                        s_sb = tp.tile([128, n * 8], F32, tag="s" + suf)
                        nc.scalar.copy(s_sb, tail[:, 8 * lo:8 * hi])
                        sv = s_sb.rearrange("p (c k) -> p c k", k=8)
                        lv, pv = sv[:, :, 4:8], sv[:, :, 0:4]
                        q1 = tp.tile([128, n * 4], F32, tag="q1" + suf)
                        nc.gpsimd.tensor_scalar(
                            q1, lv, 1.0 / 6.0, 0.5, ALU.mult, ALU.add)
                        q2 = tp.tile([128, n * 4], F32, tag="q2" + suf)
                        q2v = q2.rearrange("p (c k) -> p c k", k=4)
                        nc.gpsimd.tensor_mul(q2v, q1, lv)
                        q3 = tp.tile([128, n * 4], F32, tag="q3" + suf)
                        q3v = q3.rearrange("p (c k) -> p c k", k=4)
                        nc.gpsimd.scalar_tensor_tensor(
                            q3v, q2, 1.0, lv, ALU.add, ALU.mult)
                        expl = tp.tile([128, n * 4], F32, tag="expl" + suf)
                        nc.gpsimd.tensor_scalar(
                            expl, q3, 1.0, None, ALU.add)
                        ev = expl.rearrange("p (c k) -> p c k", k=4)
                        w_sb = tp.tile([128, n * 4], F32, tag="w" + suf)
                        wv = w_sb.rearrange("p (c k) -> p c k", k=4)
                        nc.gpsimd.tensor_mul(wv, pv, ev)
                    else:
                        expl = tp.tile([128, n * 4], F32, tag="expl" + suf)
                        ev = expl.rearrange("p (c k) -> p c k", k=4)
                        nc.scalar.activation(ev, tv[:, :, 4:8], AF.Exp)
                        w_sb = tp.tile([128, n * 4], F32, tag="w" + suf)
                        wv = w_sb.rearrange("p (c k) -> p c k", k=4)
                    if mode == "dve_drain":
                        # final drain piece: everything on DVE serially --
                        # avoids six gpsimd Q7-launch overheads on the
                        # kernel's critical exit path
                        nc.vector.tensor_mul(wv, tv[:, :, 0:4], ev)
                        num = tp.tile([128, n], F32, tag="num" + suf)
                        nc.vector.tensor_reduce(num, wv, AX.X, ALU.add)
                        den = tp.tile([128, n], F32, tag="den" + suf)
                        nc.vector.tensor_reduce(den, ev, AX.X, ALU.add)
                        rec = tp.tile([128, n], F32, tag="rec" + suf)
                        nc.vector.reciprocal(rec, den)
                        o_sb = tp.tile([128, n], F32, tag="o" + suf)
                        nc.vector.tensor_mul(o_sb, num, rec)
                        nc.sync.dma_start(out=out_d[rr][:, lo:hi], in_=o_sb)
                        return lambda: None
                    if mode == "dve_tt":
                        nc.vector.tensor_mul(wv, tv[:, :, 0:4], ev)
                    elif mode != "poly":
                        p_sb = tp.tile([128, n * 4], F32, tag="p" + suf)
                        pv = p_sb.rearrange("p (c k) -> p c k", k=4)
                        if mode == "pcopy_dve":
                            nc.vector.tensor_copy(pv, tv[:, :, 0:4])
                        else:
                            nc.scalar.copy(pv, tv[:, :, 0:4])
                        nc.gpsimd.tensor_mul(w_sb, p_sb, expl)
                    # 4-wide sums as pairwise adds (gpsimd reduce lacks X axis)
                    ws = tp.tile([128, n * 2], F32, tag="ws" + suf)
                    wsv = ws.rearrange("p (c k) -> p c k", k=2)
                    nc.gpsimd.tensor_add(wsv, wv[:, :, 0:2], wv[:, :, 2:4])
                    num = tp.tile([128, n], F32, tag="num" + suf)
                    nc.gpsimd.tensor_add(num, wsv[:, :, 0], wsv[:, :, 1])
                    es = tp.tile([128, n * 2], F32, tag="es" + suf)
                    esv = es.rearrange("p (c k) -> p c k", k=2)
                    nc.gpsimd.tensor_add(esv, ev[:, :, 0:2], ev[:, :, 2:4])
                    den = tp.tile([128, n], F32, tag="den" + suf)
                    nc.gpsimd.tensor_add(den, esv[:, :, 0], esv[:, :, 1])

                    def phase2():
                        rec = tp.tile([128, n], F32, tag="rec" + suf)
                        nc.vector.reciprocal(rec, den)
                        o_sb = tp.tile([128, n], F32, tag="o" + suf)
                        nc.gpsimd.tensor_mul(o_sb, num, rec)
                        nc.sync.dma_start(out=out_d[rr][:, lo:hi], in_=o_sb)
                    return phase2

                # staggered interleave: gate(g) leads its expert tiles by
                # ~2 tiles so logits/bias are ready early without starving
                # Act/DVE at round starts.
                def expert_tile(t):
                    preds_block(t, *stage2relu(t, stage1(t, H1_PATTERN[t])))

                def flush_pending():
                    for p2 in pending:
                        p2()
                    pending.clear()

                last = r == NR - 1
                if carry is None:
                    h1r_t0 = stage1(0, H1_PATTERN[0])
                    h1r_t1 = stage1(1, "dve")
                else:
                    h1r_t0, h1r_t1 = carry
                gate_block(0, True)
                preds_block(0, *stage2relu(0, h1r_t0))
                preds_block(1, *stage2relu(1, h1r_t1))
                gate_block(1, False)
                expert_tile(2)
                flush_pending()  # prev round's tail phase2 (data long ready)
                # LAST_PIECES: (issue_after_tile, flush_after_tile, lo, hi,
                # mode) for the final round's tail pieces
                lp = {}
                if last:
                    for i, (it, ft, lo, hi, md) in enumerate(LAST_PIECES):
                        lp.setdefault(it, []).append(("issue", lo, hi, i, md))
                        if ft is not None:
                            lp.setdefault(ft, []).append(("flush",))

                def tile_hooks(t):
                    for h in lp.get(t, ()):
                        if h[0] == "issue":
                            _, lo, hi, i, md = h
                            p2 = round_tail(lo, hi, "p%d" % i, r, mode=md)
                            if md == "dve_drain":
                                pass
                            else:
                                pending.append(p2)
                        else:
                            flush_pending()

                expert_tile(3); tile_hooks(3)
                expert_tile(4); tile_hooks(4)
                if FLUSH_TILE == 4:
                    flush_pending()
                expert_tile(5); tile_hooks(5)
                if FLUSH_TILE == 5:
                    flush_pending()
                gate_block(2, False)
                expert_tile(6); tile_hooks(6)
                # prefetch next round's x while this round is mid-flight
                if r + 1 < NR and XPF_TILE == 6:
                    x_next = xp.tile([64, RS], BF16, tag="x")
                    issue_x_dmas(r + 1, x_next)
                expert_tile(7); tile_hooks(7)
                if r + 1 < NR and XPF_TILE == 7:
                    x_next = xp.tile([64, RS], BF16, tag="x")
                    issue_x_dmas(r + 1, x_next)
                expert_tile(8); tile_hooks(8)
                if r + 1 < NR and XPF_TILE == 8:
                    x_next = xp.tile([64, RS], BF16, tag="x")
                    issue_x_dmas(r + 1, x_next)
                expert_tile(9); tile_hooks(9)
                gate_block(3, False)
                expert_tile(10); tile_hooks(10)
                expert_tile(11); tile_hooks(11)
                expert_tile(12); tile_hooks(12)
                expert_tile(13); tile_hooks(13)
                expert_tile(14); tile_hooks(14)
                expert_tile(15); tile_hooks(15)
                if last:
                    flush_pending()
                else:
                    pending.append(round_tail(0, 64, "", r))
                    # software-pipeline the next round's first two stage-1s
                    # so Act never idles across the round boundary
                    nxt0 = stage1(0, H1_PATTERN[0], xs_buf=x_next)
                    nxt1 = stage1(1, H1_PATTERN[1], xs_buf=x_next)
                    carry = (nxt0, nxt1)
                    x_cur = x_next



    if not nc.is_finalized():
        nc.finalize()
    return nc


def _pack_host(w1, b1, bn1_g, bn1_b, bn1_m, bn1_v, w2, b2, bn2_g, bn2_b,
               bn2_m, bn2_v, w3, b3, wp, bp, gw1, gb1, gw2, gb2):
    f = np.float32
    s1 = (bn1_g / np.sqrt(bn1_v + EPS)).astype(f)
    w1e = (w1 * s1[:, None, :]).astype(f)                       # (E,IN,H)
    c1 = ((b1 - bn1_m) * s1 + bn1_b).astype(f)                  # (E,H)
    s2 = (bn2_g / np.sqrt(bn2_v + EPS)).astype(f)
    w2e = (w2 * s2[:, None, :]).astype(f)                       # (E,H,H)
    c2 = ((b2 - bn2_m) * s2 + bn2_b).astype(f)                  # (E,H)
    wep = np.einsum("ehm,em->eh", w3, wp).astype(f)             # (E,H)
    bep = (np.einsum("em,em->e", b3, wp) + bp).astype(f)        # (E,)

    aw = np.abs(wep)                                            # (E,H)
    sg = np.sign(wep).astype(f)
    w2f = w2e * aw[:, None, :]                                  # cols scaled
    c2f = c2 * aw

    wb = np.zeros((128, WB_W), f)  # cols X00:X00+512 filled per core
    wb[0:IN, W1A0:W1A0 + 64] = w1e[0]
    wb[0:IN, W1A0 + 64:W1A0 + 128] = w1e[1]
    wb[0:IN, W1B0:W1B0 + 64] = w1e[2]
    wb[0:IN, W1B0 + 64:W1B0 + 128] = w1e[3]
    wb[IN, W1A0:W1A0 + 128] = np.concatenate([c1[0], c1[1]])
    wb[IN, W1B0:W1B0 + 128] = np.concatenate([c1[2], c1[3]])
    wb[0:IN, GW10:GW10 + 32] = gw1
    wb[IN, GW10:GW10 + 32] = gb1
    wb[0:64, W2A0:W2A0 + 64] = w2f[0]
    wb[64:128, W2A0 + 64:W2A0 + 128] = w2f[1]
    wb[0:64, W2B0:W2B0 + 64] = w2f[2]
    wb[64:128, W2B0 + 64:W2B0 + 128] = w2f[3]
    wb[0:64, SGA0] = sg[0]
    wb[64:128, SGA0 + 1] = sg[1]
    wb[0:64, SGB0] = sg[2]
    wb[64:128, SGB0 + 1] = sg[3]
    for gi in range(4):
        wb[32 * gi:32 * gi + 32, GW2R0:GW2R0 + 4] = gw2
    wb[IN, BEP0:BEP0 + 4] = bep
    wb[IN, BEP0 + 4:BEP0 + 8] = gb2

    wf = np.zeros((128, WF_W), f)
    wf[:, 2] = np.concatenate([c2f[0], c2f[1]])
    wf[:, 3] = np.concatenate([c2f[2], c2f[3]])
    return dict(wb=wb.astype(ml_dtypes.bfloat16), wf=wf)


def _x_core(xc):
    """(BC, 59) f32 -> (64, BC) bf16 feature-major with ones row at 59."""
    xt = np.zeros((64, BC), np.float32)
    xt[:IN] = xc.T
    xt[IN] = 1.0
    return np.ascontiguousarray(xt).astype(ml_dtypes.bfloat16)


def _unpack_out(o):
    """(NR, 128, 64) -> (BC,): row = 8192 r + 512 t + 128 q + b, col=4t+q."""
    o = np.asarray(o, np.float32).reshape(NR, 128, NT, 4)
    return np.ascontiguousarray(o.transpose(0, 2, 3, 1)).reshape(BC)


def _core_maps(xt, packed):
    wbc = packed["wb"].copy()
    wbc[0:64, X00:X00 + 512] = xt[:, 0:512]
    return {"x": xt, "wb": wbc, "wf": packed["wf"]}


def _sim_inputs(x_full, packed):
    return _core_maps(_x_core(np.asarray(x_full, np.float32)[:BC]), packed)


def kernel(**inputs):
    x = np.asarray(inputs["x"], dtype=np.float32)
    wk = {k: np.asarray(v, dtype=np.float32) for k, v in inputs.items()
          if k != "x"}
    packed = _pack_host(**wk)

    if "nc" not in _CACHE:
        _CACHE["nc"] = _build()
    nc = _CACHE["nc"]

    in_maps = [_core_maps(_x_core(x[c * BC:(c + 1) * BC]), packed)
               for c in range(NCORES)]

    res = run_bass_kernel_spmd(nc, in_maps, core_ids=list(range(NCORES)))
    _CACHE["last"] = res
    outs = [_unpack_out(r["out"]) for r in res.results]
    return np.concatenate(outs).reshape(B, 1).astype(np.float32)

